# revision 5
# baseline (speedup 1.0000x reference)
"""CFNet interaction block on 8 trn2 NeuronCores — v3 SPMD bass/tile kernel.

Strategy (per core c of 8, SPMD — one program, per-core input data):
  - Edges sharded by ATOM ranges: core c owns atoms [c*NA, (c+1)*NA) and all
    edges whose (sorted) seg_i falls there.  Outputs are disjoint; the host
    concatenates (no device collective).
  - Edges grouped by 128-atom chunk of seg_i.  Within a chunk, edges are
    reordered [idx_j < 25000 ..., idx_j >= 25000 ...], each run padded to a
    UNIFORM tile count (Lmax / Hmax tiles of 128 edges, data-derived max over
    all cores+chunks).  Padding slots are dump edges: dijk-cols = 0 ->
    w = ssp(ssp(0)@W2) = 0 exactly, and their gather idx points at a zeroed
    row of the feature table, so they contribute nothing.
  - The low/high split keeps SWDGE gather indices within int16: the feature
    table xf_dram [50177, 128] bf16 holds row 0 = zeros, rows 1..50000 =
    xf = x @ Win (computed on device), rows 50001.. = zeros.  Low calls
    gather rows [0..25000] (idx = j+1), high calls gather from the slice
    starting at row 25001 (idx = j-25000; zero pad idx = 25000).  256B/row
    single-packet gathers - half the HBM traffic of the old 512B pair
    gathers, and no DVE parity-select is needed.
  - dijk is uploaded centered and fp8: ddr [128, 2, E_PAD] float8_e4m3 holds
    (dijk - 0.5) for k = 0..255 in DoubleRow operand layout (k = 2p+r), and
    d45 [45, E_PAD] holds k = 256..299 plus a constant-ones row.  mm1 =
    2 DoubleRow matmuls (fp8 hi + e5m2 residual of W1) + one 45-row bf16
    matmul whose last row adds 0.5*colsum(W1), undoing the centering
    exactly.  Centering halves the fp8 quantization error of dijk.
  - Per pair of 512-edge blocks: mm1 -> Exp (per 1024-edge PSUM pair-tile),
    Ln(0.5 + 0.5 e) batched per 2048-edge quad; mm2 per 128-edge tile;
    Exp per pair, Ln per quad -> w; wf = w * f (DVE, bf16); one-hot S via
    iota == cid (DVE); conv^T[f,a] += wf_t^T @ S_t accumulated in PSUM over
    the chunk's TPC tiles.
  - Tail per chunk (pipelined): z3^T = Wout^T conv, ssp, v = h @ Wd,
    y = x + v.
"""

import math
import sys

import numpy as np
import ml_dtypes

sys.path.insert(0, "/opt/trn_rl_repo")

import concourse.bacc as bacc
import concourse.bass as bass
import concourse.mybir as mybir
from concourse import tile
from concourse.bass_utils import run_bass_kernel_spmd

dt = mybir.dt
AF = mybir.ActivationFunctionType
ALU = mybir.AluOpType
BF16 = ml_dtypes.bfloat16
E4M3 = mybir.dt.np(dt.float8e4)
E5M2 = mybir.dt.np(dt.float8e5)

N_CORES = 8
ACH = 128            # atoms per conv chunk
HALF = 25000         # atom-id split for int16 gather range
GCT = 8              # max tiles (x128 idx) per dma_gather call (ring = 1024)
DIJK_FP8 = True


def _ceil(a, b):
    return -(-a // b)


class Plan:
    def __init__(self, n_atoms, n_edges, n_in, lmax, hmax):
        assert n_atoms % N_CORES == 0 and n_in == 300
        self.n_atoms, self.n_edges, self.n_in = n_atoms, n_edges, n_in
        self.NA = n_atoms // N_CORES
        self.NCH = _ceil(self.NA, ACH)
        self.NA_PAD = self.NCH * ACH
        self.Lmax, self.Hmax = lmax, hmax
        self.TPC = lmax + hmax
        self.T = self.NCH * self.TPC
        self.E_PAD = self.T * 128
        # xf table: row 0 zero, rows 1..n_atoms = atoms, padding zero rows
        self.NSG = _ceil(n_atoms, 512)            # write groups of 512 rows
        self.NXF = 1 + self.NSG * 512             # xf_dram rows
        self.NXT = self.NSG * 512                 # xT columns
        # gather call layout within a chunk: (row_off, ntiles, is_high)
        calls = []
        for t0 in range(0, lmax, GCT):
            calls.append((t0, min(GCT, lmax - t0), 0))
        for t0 in range(0, hmax, GCT):
            calls.append((lmax + t0, min(GCT, hmax - t0), 1))
        self.calls = calls


def _plan_from_data(n_atoms, n_edges, n_in, idx_j, seg_i):
    na = n_atoms // N_CORES
    nch = _ceil(na, ACH)
    seg = np.asarray(seg_i).astype(np.int64)
    idxj = np.asarray(idx_j).astype(np.int64)
    bounds = np.searchsorted(seg, np.arange(N_CORES + 1) * na)
    lmax = hmax = 1
    for c in range(N_CORES):
        lo, hi = int(bounds[c]), int(bounds[c + 1])
        if lo == hi:
            continue
        ch = (seg[lo:hi] - c * na) // ACH
        high = (idxj[lo:hi] >= HALF).astype(np.int64)
        cnt = np.bincount(ch * 2 + high, minlength=nch * 2)
        lmax = max(lmax, _ceil(int(cnt[0::2].max()), 128))
        hmax = max(hmax, _ceil(int(cnt[1::2].max()), 128))
    return Plan(n_atoms, n_edges, n_in, lmax, hmax), bounds


def _wrap_idx(idx1d):
    """[n] int -> [128, n//16] int16 SWDGE layout (16-wrapped, tiled x8)."""
    n = idx1d.shape[0]
    w = idx1d.astype(np.int16).reshape(n // 16, 16).T
    return np.tile(w, (8, 1))


def shard_inputs(p, dijk, idx_j, seg_i, x, bounds):
    seg = np.asarray(seg_i).astype(np.int64)
    idxj = np.asarray(idx_j).astype(np.int64)
    n_in = p.n_in
    per_core = []
    for c in range(N_CORES):
        lo, hi = int(bounds[c]), int(bounds[c + 1])
        es = seg[lo:hi] - c * p.NA
        ej = idxj[lo:hi]
        n = hi - lo
        ch = es // ACH
        high = (ej >= HALF).astype(np.int64)
        gid = ch * 2 + high
        perm = np.argsort(gid, kind="stable")
        gid_s = gid[perm]
        sizes = np.bincount(gid, minlength=p.NCH * 2)
        first = np.zeros(p.NCH * 2, dtype=np.int64)
        first[1:] = np.cumsum(sizes)[:-1]
        base = np.empty(p.NCH * 2, dtype=np.int64)
        base[0::2] = np.arange(p.NCH) * p.TPC * 128
        base[1::2] = np.arange(p.NCH) * p.TPC * 128 + p.Lmax * 128
        dst = base[gid_s] + (np.arange(n) - first[gid_s])

        es_s, ej_s, high_s = es[perm], ej[perm], high[perm]

        # dijk stream, centered, with const row
        dsrc = np.asarray(dijk[lo:hi], dtype=np.float32)[perm] - 0.5
        if DIJK_FP8:
            dstream = np.zeros((p.E_PAD, 256), dtype=E4M3)
            dstream[dst] = dsrc[:, :256].astype(E4M3)
            ddr = np.ascontiguousarray(
                dstream.reshape(p.E_PAD, 128, 2).transpose(1, 2, 0)
            )  # [128, 2, E_PAD]: (p, r) -> k = 2p + r
            d45 = np.zeros((45, p.E_PAD), dtype=E4M3)
            d45[:44, dst] = dsrc[:, 256:300].astype(E4M3).T
            d45[44, dst] = np.ones(n, dtype=E4M3)
        else:
            dstream = np.zeros((p.E_PAD, 256), dtype=BF16)
            dstream[dst] = dsrc[:, :256].astype(BF16)
            ddr = np.ascontiguousarray(
                dstream.reshape(p.E_PAD, 2, 128).transpose(2, 1, 0)
            )  # [128, 2, E_PAD]: (p, s) -> k = s*128 + p
            d45 = np.zeros((45, p.E_PAD), dtype=BF16)
            d45[:44, dst] = dsrc[:, 256:300].astype(BF16).T
            d45[44, dst] = np.ones(n, dtype=BF16)
        del dstream, dsrc

        cidv = np.zeros(p.E_PAD, dtype=np.float32)
        cidv[dst] = (es_s % ACH).astype(np.float32)
        cid_tbl = np.ascontiguousarray(
            cidv.reshape(p.T, 128).T.astype(BF16)
        )  # [128, T]

        gv = np.zeros((p.NCH, p.TPC, 128), dtype=np.int64)
        gv[:, p.Lmax:, :] = HALF            # high-region padding -> zero row
        gv = gv.reshape(-1)
        gv[dst[high_s == 0]] = ej_s[high_s == 0] + 1
        gv[dst[high_s == 1]] = ej_s[high_s == 1] - HALF
        fidx = np.zeros((128, p.T * 8), dtype=np.int16)
        for ci in range(p.NCH):
            for (t0, nt, _hi) in p.calls:
                s0 = (ci * p.TPC + t0) * 128
                col = (ci * p.TPC + t0) * 8
                fidx[:, col: col + nt * 8] = _wrap_idx(gv[s0: s0 + nt * 128])

        per_core.append(
            dict(
                ddr=ddr,
                d45=np.ascontiguousarray(d45),
                cid=cid_tbl,
                fidx=fidx,
                xslice=_pad_rows(x[c * p.NA:(c + 1) * p.NA], p.NA_PAD),
            )
        )
    return per_core


def _pad_rows(a, n):
    out = np.zeros((n,) + a.shape[1:], dtype=np.asarray(a).dtype)
    out[: a.shape[0]] = np.asarray(a)
    return out


def build_program(p):
    # Force one activation table (Exp+Ln coexist in natural_log_exp_and_others)
    import concourse.bacc as _bacc_mod
    _orig_gat = _bacc_mod.get_activation_tables

    def _one_table(arch):
        t = _orig_gat(arch)
        keep = "natural_log_exp_and_others"
        assert keep in t
        return {k: (v if k == keep else set()) for k, v in t.items()}

    _bacc_mod.get_activation_tables = _one_table
    try:
        return _build_program_inner(p)
    finally:
        _bacc_mod.get_activation_tables = _orig_gat


def _build_program_inner(p):
    nc = bacc.Bacc(None, target_bir_lowering=False, num_swdge_queues=4)
    DR = mybir.MatmulPerfMode.DoubleRow
    f8 = dt.float8e4 if DIJK_FP8 else dt.bfloat16

    # ---- dram parameters ----
    xT = nc.declare_dram_parameter("xT", [128, p.NXT], dt.bfloat16, isOutput=False)
    xslice = nc.declare_dram_parameter("xslice", [p.NA_PAD, 128], dt.float32, isOutput=False)
    ddr = nc.declare_dram_parameter("ddr", [128, 2, p.E_PAD], f8, isOutput=False)
    d45 = nc.declare_dram_parameter("d45", [45, p.E_PAD], f8, isOutput=False)
    fidx = nc.declare_dram_parameter("fidx", [128, p.T * 8], dt.int16, isOutput=False)
    cid = nc.declare_dram_parameter("cid", [128, p.T], dt.bfloat16, isOutput=False)
    if DIJK_FP8:
        w1a = nc.declare_dram_parameter("w1a", [128, 2, 128], dt.float8e4, isOutput=False)
        w1r = nc.declare_dram_parameter("w1r", [128, 2, 128], dt.float8e5, isOutput=False)
    else:
        w1a = nc.declare_dram_parameter("w1a", [128, 128], dt.bfloat16, isOutput=False)
        w1r = nc.declare_dram_parameter("w1r", [128, 128], dt.bfloat16, isOutput=False)
    w145 = nc.declare_dram_parameter("w145", [45, 128], dt.bfloat16, isOutput=False)
    w2b = nc.declare_dram_parameter("w2b", [128, 128], dt.bfloat16, isOutput=False)
    winb = nc.declare_dram_parameter("winb", [128, 128], dt.bfloat16, isOutput=False)
    woutb = nc.declare_dram_parameter("woutb", [128, 128], dt.bfloat16, isOutput=False)
    wdb = nc.declare_dram_parameter("wdb", [128, 128], dt.bfloat16, isOutput=False)
    iota = nc.declare_dram_parameter("iota", [128, 4, 128], dt.bfloat16, isOutput=False)

    y_out = nc.declare_dram_parameter("y_out", [p.NA_PAD, 128], dt.float32, isOutput=True)
    v_out = nc.declare_dram_parameter("v_out", [p.NA_PAD, 128], dt.float32, isOutput=True)

    # ---- internal dram: feature table (row 0 zero, rows 1.. = xf) ----
    xf_dram = nc.dram_tensor("xf_dram", [p.NXF, 128], dt.bfloat16)

    with tile.TileContext(nc) as tc:
        with (
            tc.tile_pool(name="const", bufs=1) as constp,
            tc.tile_pool(name="xtp", bufs=2) as xtp,
            tc.tile_pool(name="xfp", bufs=2) as xfp,
            tc.tile_pool(name="dld", bufs=2) as dld,
            tc.tile_pool(name="fbp", bufs=2) as fbp,
            tc.tile_pool(name="idxp", bufs=4) as idxp,
            tc.tile_pool(name="eb", bufs=2) as eb,
            tc.tile_pool(name="sgp", bufs=2) as sgp,
            tc.tile_pool(name="tailp", bufs=2) as tailp,
            tc.tile_pool(name="psum", bufs=2, space="PSUM") as psum,
        ):
            # ---- constants ----
            if DIJK_FP8:
                w1a_sb = constp.tile([128, 2, 128], dt.float8e4)
                nc.sync.dma_start(out=w1a_sb[:], in_=w1a[:, :, :])
                w1r_sb = constp.tile([128, 2, 128], dt.float8e5)
                nc.sync.dma_start(out=w1r_sb[:], in_=w1r[:, :, :])
            else:
                w1a_sb = constp.tile([128, 128], dt.bfloat16)
                nc.sync.dma_start(out=w1a_sb[:], in_=w1a[:, :])
                w1r_sb = constp.tile([128, 128], dt.bfloat16)
                nc.sync.dma_start(out=w1r_sb[:], in_=w1r[:, :])
            w145_sb = constp.tile([45, 128], dt.bfloat16)
            nc.sync.dma_start(out=w145_sb[:], in_=w145[:, :])
            w2sb = constp.tile([128, 128], dt.bfloat16)
            nc.sync.dma_start(out=w2sb[:], in_=w2b[:, :])
            winsb = constp.tile([128, 128], dt.bfloat16)
            nc.sync.dma_start(out=winsb[:], in_=winb[:, :])
            woutsb = constp.tile([128, 128], dt.bfloat16)
            nc.sync.dma_start(out=woutsb[:], in_=woutb[:, :])
            wdsb = constp.tile([128, 128], dt.bfloat16)
            nc.sync.dma_start(out=wdsb[:], in_=wdb[:, :])
            iota4_sb = constp.tile([128, 4, 128], dt.bfloat16)
            nc.sync.dma_start(out=iota4_sb[:], in_=iota[:, :, :])
            cid_sb = constp.tile([128, p.T], dt.bfloat16)
            nc.sync.dma_start(out=cid_sb[:], in_=cid[:, :])
            half_c = constp.tile([128, 1], dt.float32)
            nc.gpsimd.memset(half_c[:], 0.5)
            zrow = constp.tile([1, 128], dt.bfloat16)
            nc.gpsimd.memset(zrow[:], 0.0)
            nc.sync.dma_start(out=xf_dram[0:1, :], in_=zrow[:])

            # ---- phase 0: xf = x @ Win -> xf_dram rows 1.. ----
            xts = None
            for sg in range(p.NSG):
                if sg % 4 == 0:
                    wdt = min(2048, p.NXT - sg * 512)
                    xts = xtp.tile([128, 2048], dt.bfloat16, tag="xts")
                    nc.sync.dma_start(
                        out=xts[:, :wdt],
                        in_=xT[:, sg * 512: sg * 512 + wdt],
                    )
                xf_ps = psum.tile([128, 4, 128], dt.float32, tag="z2")
                o = (sg % 4) * 512
                for jj in range(4):
                    nc.tensor.matmul(
                        xf_ps[:, jj, :],
                        xts[:, o + jj * 128: o + (jj + 1) * 128],
                        winsb[:],
                        start=True,
                        stop=True,
                    )
                xf_sb = xfp.tile([128, 4, 128], dt.bfloat16, tag="xfsb")
                nc.vector.tensor_copy(xf_sb[:], xf_ps[:])
                nc.sync.dma_start(
                    out=xf_dram[1 + sg * 512: 1 + sg * 512 + 512, :].rearrange(
                        "(j pp) f -> pp j f", pp=128
                    ),
                    in_=xf_sb[:],
                )

            # ---- tail emitter (per 128-atom chunk) ----
            def emit_tail(c, cps):
                cT = tailp.tile([128, 128], dt.bfloat16, tag="cT")
                nc.vector.tensor_copy(cT[:], cps[:])
                z3_ps = psum.tile([128, 128], dt.float32, tag="tail", bufs=1)
                nc.tensor.matmul(z3_ps[:], woutsb[:], cT[:], start=True, stop=True)
                e3 = tailp.tile([128, 128], dt.float32, tag="e3")
                nc.scalar.activation(e3[:], z3_ps[:], AF.Exp)
                hT = tailp.tile([128, 128], dt.bfloat16, tag="hT")
                nc.scalar.activation(
                    hT[:], e3[:], AF.Ln, bias=half_c[:], scale=half_c[:]
                )
                v_ps = psum.tile([128, 128], dt.float32, tag="tail", bufs=1)
                nc.tensor.matmul(v_ps[:], hT[:], wdsb[:], start=True, stop=True)
                v_sb = tailp.tile([128, 128], dt.float32, tag="v")
                nc.vector.tensor_copy(v_sb[:], v_ps[:])
                nc.sync.dma_start(
                    out=v_out[c * 128:(c + 1) * 128, :], in_=v_sb[:]
                )
                xs = tailp.tile([128, 128], dt.float32, tag="xs")
                nc.sync.dma_start(
                    out=xs[:], in_=xslice[c * 128:(c + 1) * 128, :]
                )
                y_sb = tailp.tile([128, 128], dt.float32, tag="y")
                nc.vector.tensor_tensor(y_sb[:], v_sb[:], xs[:], ALU.add)
                nc.sync.dma_start(
                    out=y_out[c * 128:(c + 1) * 128, :], in_=y_sb[:]
                )

            # ---- edge phase, per chunk ----
            TPC = p.TPC
            NB = _ceil(TPC, 4)                  # 512-edge blocks per chunk
            NPAIR = _ceil(NB, 2)
            gq = 0
            xf_low = xf_dram[0: HALF + 1, :]
            xf_high = xf_dram[HALF + 1: p.NXF, :]

            def mm1(t1_ps, psl, dq, d45t, b, ne):
                e0 = b * 512
                if DIJK_FP8:
                    nc.tensor.matmul(
                        t1_ps[:, psl], w1a_sb[:], dq[:, :, e0:e0 + ne],
                        start=True, stop=False, perf_mode=DR,
                    )
                    nc.tensor.matmul(
                        t1_ps[:, psl], w1r_sb[:], dq[:, :, e0:e0 + ne],
                        start=False, stop=False, perf_mode=DR,
                    )
                else:
                    nc.tensor.matmul(
                        t1_ps[:, psl], w1a_sb[:], dq[:, 0, e0:e0 + ne],
                        start=True, stop=False,
                    )
                    nc.tensor.matmul(
                        t1_ps[:, psl], w1r_sb[:], dq[:, 1, e0:e0 + ne],
                        start=False, stop=False,
                    )
                nc.tensor.matmul(
                    t1_ps[:, psl], w145_sb[:], d45t[:, e0:e0 + ne],
                    start=False, stop=True,
                )

            for ci in range(p.NCH):
                e0c = ci * TPC * 128
                dq = dld.tile([128, 2, TPC * 128], f8, tag="dq")
                nc.sync.dma_start(out=dq[:], in_=ddr[:, :, e0c:e0c + TPC * 128])
                d45t = dld.tile([45, TPC * 128], f8, tag="d45")
                nc.sync.dma_start(out=d45t[:], in_=d45[:, e0c:e0c + TPC * 128])
                fgat = fbp.tile([128, TPC, 128], dt.bfloat16, tag="fgat")
                for (t0, nt, is_high) in p.calls:
                    nidx = nt * 128
                    idxt = idxp.tile([128, GCT * 8], dt.int16, tag="idx")
                    col = (ci * TPC + t0) * 8
                    nc.gpsimd.dma_start(
                        out=idxt[:, : nt * 8], in_=fidx[:, col: col + nt * 8]
                    )
                    nc.gpsimd.dma_gather(
                        fgat[:, t0:t0 + nt, :],
                        xf_high if is_high else xf_low,
                        idxt[:, : nt * 8], nidx, nidx, 128,
                        single_packet=True, queue_num=gq % 4,
                    )
                    gq += 1

                conv_ps = None
                for pr in range(NPAIR):
                    b0 = pr * 2
                    nblk = min(2, NB - b0)
                    ntl = [min(4, TPC - 4 * (b0 + i)) for i in range(nblk)]
                    ntt = sum(ntl)
                    # mm1 for the pair
                    t1_ps = psum.tile([128, 1024], dt.float32, tag="t1")
                    for i in range(nblk):
                        mm1(t1_ps, slice(i * 512, i * 512 + ntl[i] * 128),
                            dq, d45t, b0 + i, ntl[i] * 128)
                    # Exp over the pair (from PSUM), Ln over the pair
                    e1q = eb.tile([128, 8, 128], dt.float32, tag="e1")
                    t1sq = eb.tile([128, 8, 128], dt.bfloat16, tag="t1s")
                    for i in range(nblk):
                        ne = ntl[i] * 128
                        nc.scalar.activation(
                            e1q[:, i * 4: i * 4 + ntl[i], :],
                            t1_ps[:, i * 512: i * 512 + ne].rearrange(
                                "pp (i f) -> pp i f", f=128
                            ),
                            AF.Exp,
                        )
                    lnsl = (
                        slice(0, 4 + ntl[1]) if nblk == 2 and ntl[0] == 4
                        else slice(0, ntl[0])
                    )
                    nc.scalar.activation(
                        t1sq[:, lnsl, :], e1q[:, lnsl, :], AF.Ln,
                        bias=half_c[:], scale=half_c[:],
                    )
                    # mm2 + Exp per block, Ln per pair
                    e2q = eb.tile([128, 8, 128], dt.float32, tag="e2")
                    wq = eb.tile([128, 8, 128], dt.bfloat16, tag="w")
                    for i in range(nblk):
                        z2_ps = psum.tile([128, 4, 128], dt.float32, tag="z2")
                        for t in range(ntl[i]):
                            nc.tensor.matmul(
                                z2_ps[:, t, :],
                                t1sq[:, i * 4 + t, :],
                                w2sb[:],
                                start=True, stop=True,
                            )
                        nc.scalar.activation(
                            e2q[:, i * 4: i * 4 + ntl[i], :],
                            z2_ps[:, : ntl[i], :],
                            AF.Exp,
                        )
                    nc.scalar.activation(
                        wq[:, lnsl, :], e2q[:, lnsl, :], AF.Ln,
                        bias=half_c[:], scale=half_c[:],
                    )
                    # wf, S, conv accumulation
                    for i in range(nblk):
                        b = b0 + i
                        nt = ntl[i]
                        t0 = b * 4
                        wf = sgp.tile([128, 4, 128], dt.bfloat16, tag="wf")
                        nc.vector.tensor_tensor(
                            wf[:, :nt, :],
                            wq[:, i * 4: i * 4 + nt, :],
                            fgat[:, t0:t0 + nt, :],
                            ALU.mult,
                        )
                        S_blk = sgp.tile([128, 4, 128], dt.bfloat16, tag="S")
                        nc.vector.tensor_tensor(
                            S_blk[:, :nt, :],
                            iota4_sb[:, :nt, :],
                            cid_sb[:, ci * TPC + t0: ci * TPC + t0 + nt]
                            .to_broadcast([128, nt, 128]),
                            ALU.is_equal,
                        )
                        for t in range(nt):
                            k = t0 + t
                            if k == 0:
                                conv_ps = psum.tile(
                                    [128, 128], dt.float32, tag="conv", bufs=1
                                )
                            nc.tensor.matmul(
                                conv_ps[:], wf[:, t, :], S_blk[:, t, :],
                                start=(k == 0), stop=(k == TPC - 1),
                            )
                emit_tail(ci, conv_ps)

    nc.finalize()
    return nc


_PROG_CACHE = {}


def kernel(x, dijk, W1, b1, W2, b2, Win, Wout, bout, Wd, bd, idx_j, seg_i, seg_j):
    x = np.ascontiguousarray(np.asarray(x, dtype=np.float32))
    for b in (b1, b2, bout, bd):
        assert np.abs(np.asarray(b)).max() == 0.0, "nonzero biases unsupported"

    n_atoms, n_basis = x.shape
    n_edges, n_in = np.asarray(dijk).shape
    assert n_basis == 128 and np.asarray(W2).shape == (128, 128)

    p, bounds = _plan_from_data(n_atoms, n_edges, n_in, idx_j, seg_i)
    per_core = shard_inputs(p, dijk, idx_j, seg_i, x, bounds)

    key = (n_atoms, n_edges, n_in, p.Lmax, p.Hmax, DIJK_FP8)
    if key not in _PROG_CACHE:
        _PROG_CACHE[key] = build_program(p)
    nc = _PROG_CACHE[key]

    W1f = np.asarray(W1, dtype=np.float32)
    if DIJK_FP8:
        w1hi = W1f[:256].astype(E4M3)
        w1res = (W1f[:256] - w1hi.astype(np.float32)).astype(E5M2)
        w1a_h = np.ascontiguousarray(w1hi.reshape(128, 2, 128))
        w1r_h = np.ascontiguousarray(w1res.reshape(128, 2, 128))
    else:
        w1a_h = np.ascontiguousarray(W1f[0:128].astype(BF16))
        w1r_h = np.ascontiguousarray(W1f[128:256].astype(BF16))
    w145_h = np.zeros((45, 128), dtype=BF16)
    w145_h[:44] = W1f[256:300].astype(BF16)
    w145_h[44] = (0.5 * W1f.sum(axis=0)).astype(BF16)

    xTh = np.zeros((128, p.NXT), dtype=BF16)
    xTh[:, :n_atoms] = x.T
    common = dict(
        xT=xTh,
        w1a=w1a_h,
        w1r=w1r_h,
        w145=w145_h,
        w2b=np.asarray(W2, dtype=np.float32).astype(BF16),
        winb=np.asarray(Win, dtype=np.float32).astype(BF16),
        woutb=np.asarray(Wout, dtype=np.float32).astype(BF16),
        wdb=np.asarray(Wd, dtype=np.float32).astype(BF16),
        iota=np.tile(np.arange(128, dtype=np.float32).astype(BF16), (128, 4, 1)),
    )
    in_maps = [{**common, **pc} for pc in per_core]
    res = run_bass_kernel_spmd(nc, in_maps, list(range(N_CORES)))
    global LAST_RESULTS
    LAST_RESULTS = res

    y = np.empty((n_atoms, 128), dtype=np.float32)
    v = np.empty((n_atoms, 128), dtype=np.float32)
    for c in range(N_CORES):
        y[c * p.NA:(c + 1) * p.NA] = res.results[c]["y_out"][: p.NA]
        v[c * p.NA:(c + 1) * p.NA] = res.results[c]["v_out"][: p.NA]
    return (y, v)


# revision 6
# speedup vs baseline: 1.2345x; 1.2345x over previous
"""CFNet interaction block on 8 trn2 NeuronCores — v3 SPMD bass/tile kernel.

Strategy (per core c of 8, SPMD — one program, per-core input data):
  - Edges sharded by ATOM ranges: core c owns atoms [c*NA, (c+1)*NA) and all
    edges whose (sorted) seg_i falls there.  Outputs are disjoint; the host
    concatenates (no device collective).
  - Edges grouped by 128-atom chunk of seg_i.  Within a chunk, edges are
    reordered [idx_j < 25000 ..., idx_j >= 25000 ...], each run padded to a
    UNIFORM tile count (Lmax / Hmax tiles of 128 edges, data-derived max over
    all cores+chunks).  Padding slots are dump edges: dijk-cols = 0 ->
    w = ssp(ssp(0)@W2) = 0 exactly, and their gather idx points at a zeroed
    row of the feature table, so they contribute nothing.
  - The low/high split keeps SWDGE gather indices within int16: the feature
    table xf_dram [50177, 128] bf16 holds row 0 = zeros, rows 1..50000 =
    xf = x @ Win (computed on device), rows 50001.. = zeros.  Low calls
    gather rows [0..25000] (idx = j+1), high calls gather from the slice
    starting at row 25001 (idx = j-25000; zero pad idx = 25000).  256B/row
    single-packet gathers - half the HBM traffic of the old 512B pair
    gathers, and no DVE parity-select is needed.
  - dijk is uploaded centered and fp8: ddr [128, 2, E_PAD] float8_e4m3 holds
    (dijk - 0.5) for k = 0..255 in DoubleRow operand layout (k = 2p+r), and
    d45 [45, E_PAD] holds k = 256..299 plus a constant-ones row.  mm1 =
    2 DoubleRow matmuls (fp8 hi + e5m2 residual of W1) + one 45-row bf16
    matmul whose last row adds 0.5*colsum(W1), undoing the centering
    exactly.  Centering halves the fp8 quantization error of dijk.
  - Per pair of 512-edge blocks: mm1 -> Exp (per 1024-edge PSUM pair-tile),
    Ln(0.5 + 0.5 e) batched per 2048-edge quad; mm2 per 128-edge tile;
    Exp per pair, Ln per quad -> w; wf = w * f (DVE, bf16); one-hot S via
    iota == cid (DVE); conv^T[f,a] += wf_t^T @ S_t accumulated in PSUM over
    the chunk's TPC tiles.
  - Tail per chunk (pipelined): z3^T = Wout^T conv, ssp, v = h @ Wd,
    y = x + v.
"""

import math
import sys

import numpy as np
import ml_dtypes

sys.path.insert(0, "/opt/trn_rl_repo")

import concourse.bacc as bacc
import concourse.bass as bass
import concourse.mybir as mybir
from concourse import tile
from concourse.bass_utils import run_bass_kernel_spmd

dt = mybir.dt
AF = mybir.ActivationFunctionType
ALU = mybir.AluOpType
BF16 = ml_dtypes.bfloat16
E4M3 = mybir.dt.np(dt.float8e4)
E5M2 = mybir.dt.np(dt.float8e5)

N_CORES = 8
ACH = 128            # atoms per conv chunk
HALF = 25000         # atom-id split for int16 gather range
GCT = 8              # max tiles (x128 idx) per dma_gather call (ring = 1024)
DIJK_FP8 = True


def _ceil(a, b):
    return -(-a // b)


class Plan:
    def __init__(self, n_atoms, n_edges, n_in, lmax, hmax):
        assert n_atoms % N_CORES == 0 and n_in == 300
        self.n_atoms, self.n_edges, self.n_in = n_atoms, n_edges, n_in
        self.NA = n_atoms // N_CORES
        self.NCH = _ceil(self.NA, ACH)
        self.NA_PAD = self.NCH * ACH
        self.Lmax, self.Hmax = lmax, hmax
        self.TPC = lmax + hmax
        self.T = self.NCH * self.TPC
        self.E_PAD = self.T * 128
        # xf table: row 0 zero, rows 1..n_atoms = atoms, padding zero rows
        self.NSG = _ceil(n_atoms, 512)            # write groups of 512 rows
        self.NXF = 1 + self.NSG * 512             # xf_dram rows
        self.NXT = self.NSG * 512                 # xT columns
        # gather call layout within a chunk: (row_off, ntiles, is_high)
        calls = []
        for t0 in range(0, lmax, GCT):
            calls.append((t0, min(GCT, lmax - t0), 0))
        for t0 in range(0, hmax, GCT):
            calls.append((lmax + t0, min(GCT, hmax - t0), 1))
        self.calls = calls


def _plan_from_data(n_atoms, n_edges, n_in, idx_j, seg_i):
    na = n_atoms // N_CORES
    nch = _ceil(na, ACH)
    seg = np.asarray(seg_i).astype(np.int64)
    idxj = np.asarray(idx_j).astype(np.int64)
    bounds = np.searchsorted(seg, np.arange(N_CORES + 1) * na)
    lmax = hmax = 1
    for c in range(N_CORES):
        lo, hi = int(bounds[c]), int(bounds[c + 1])
        if lo == hi:
            continue
        ch = (seg[lo:hi] - c * na) // ACH
        high = (idxj[lo:hi] >= HALF).astype(np.int64)
        cnt = np.bincount(ch * 2 + high, minlength=nch * 2)
        lmax = max(lmax, _ceil(int(cnt[0::2].max()), 128))
        hmax = max(hmax, _ceil(int(cnt[1::2].max()), 128))
    return Plan(n_atoms, n_edges, n_in, lmax, hmax), bounds


def _wrap_idx(idx1d):
    """[n] int -> [128, n//16] int16 SWDGE layout (16-wrapped, tiled x8)."""
    n = idx1d.shape[0]
    w = idx1d.astype(np.int16).reshape(n // 16, 16).T
    return np.tile(w, (8, 1))


def shard_inputs(p, dijk, idx_j, seg_i, x, bounds):
    seg = np.asarray(seg_i).astype(np.int64)
    idxj = np.asarray(idx_j).astype(np.int64)
    n_in = p.n_in
    per_core = []
    for c in range(N_CORES):
        lo, hi = int(bounds[c]), int(bounds[c + 1])
        es = seg[lo:hi] - c * p.NA
        ej = idxj[lo:hi]
        n = hi - lo
        ch = es // ACH
        high = (ej >= HALF).astype(np.int64)
        gid = ch * 2 + high
        perm = np.argsort(gid, kind="stable")
        gid_s = gid[perm]
        sizes = np.bincount(gid, minlength=p.NCH * 2)
        first = np.zeros(p.NCH * 2, dtype=np.int64)
        first[1:] = np.cumsum(sizes)[:-1]
        base = np.empty(p.NCH * 2, dtype=np.int64)
        base[0::2] = np.arange(p.NCH) * p.TPC * 128
        base[1::2] = np.arange(p.NCH) * p.TPC * 128 + p.Lmax * 128
        dst = base[gid_s] + (np.arange(n) - first[gid_s])

        es_s, ej_s, high_s = es[perm], ej[perm], high[perm]

        # dijk stream, centered, with const row
        dsrc = np.asarray(dijk[lo:hi], dtype=np.float32)[perm] - 0.5
        if DIJK_FP8:
            dstream = np.zeros((p.E_PAD, 256), dtype=E4M3)
            dstream[dst] = dsrc[:, :256].astype(E4M3)
            ddr = np.ascontiguousarray(
                dstream.reshape(p.E_PAD, 128, 2).transpose(1, 2, 0)
            )  # [128, 2, E_PAD]: (p, r) -> k = 2p + r
            d45 = np.zeros((45, p.E_PAD), dtype=E4M3)
            d45[:44, dst] = dsrc[:, 256:300].astype(E4M3).T
            d45[44, dst] = np.ones(n, dtype=E4M3)
        else:
            dstream = np.zeros((p.E_PAD, 256), dtype=BF16)
            dstream[dst] = dsrc[:, :256].astype(BF16)
            ddr = np.ascontiguousarray(
                dstream.reshape(p.E_PAD, 2, 128).transpose(2, 1, 0)
            )  # [128, 2, E_PAD]: (p, s) -> k = s*128 + p
            d45 = np.zeros((45, p.E_PAD), dtype=BF16)
            d45[:44, dst] = dsrc[:, 256:300].astype(BF16).T
            d45[44, dst] = np.ones(n, dtype=BF16)
        del dstream, dsrc

        cidv = np.zeros(p.E_PAD, dtype=np.float32)
        cidv[dst] = (es_s % ACH).astype(np.float32)
        cid_tbl = np.ascontiguousarray(
            cidv.reshape(p.T, 128).T.astype(BF16)
        )  # [128, T]

        gv = np.zeros((p.NCH, p.TPC, 128), dtype=np.int64)
        gv[:, p.Lmax:, :] = HALF            # high-region padding -> zero row
        gv = gv.reshape(-1)
        gv[dst[high_s == 0]] = ej_s[high_s == 0] + 1
        gv[dst[high_s == 1]] = ej_s[high_s == 1] - HALF
        fidx = np.zeros((128, p.T * 8), dtype=np.int16)
        for ci in range(p.NCH):
            for (t0, nt, _hi) in p.calls:
                s0 = (ci * p.TPC + t0) * 128
                col = (ci * p.TPC + t0) * 8
                fidx[:, col: col + nt * 8] = _wrap_idx(gv[s0: s0 + nt * 128])

        per_core.append(
            dict(
                ddr=ddr,
                d45=np.ascontiguousarray(d45),
                cid=cid_tbl,
                fidx=fidx,
                xslice=_pad_rows(x[c * p.NA:(c + 1) * p.NA], p.NA_PAD),
            )
        )
    return per_core


def _pad_rows(a, n):
    out = np.zeros((n,) + a.shape[1:], dtype=np.asarray(a).dtype)
    out[: a.shape[0]] = np.asarray(a)
    return out


def build_program(p):
    # Force one activation table (Exp+Ln coexist in natural_log_exp_and_others)
    import concourse.bacc as _bacc_mod
    _orig_gat = _bacc_mod.get_activation_tables

    def _one_table(arch):
        t = _orig_gat(arch)
        keep = "natural_log_exp_and_others"
        assert keep in t
        return {k: (v if k == keep else set()) for k, v in t.items()}

    _bacc_mod.get_activation_tables = _one_table
    try:
        return _build_program_inner(p)
    finally:
        _bacc_mod.get_activation_tables = _orig_gat


def _build_program_inner(p):
    nc = bacc.Bacc(None, target_bir_lowering=False, num_swdge_queues=4)
    DR = mybir.MatmulPerfMode.DoubleRow
    f8 = dt.float8e4 if DIJK_FP8 else dt.bfloat16

    # ---- dram parameters ----
    xT = nc.declare_dram_parameter("xT", [128, p.NXT], dt.bfloat16, isOutput=False)
    xslice = nc.declare_dram_parameter("xslice", [p.NA_PAD, 128], dt.float32, isOutput=False)
    ddr = nc.declare_dram_parameter("ddr", [128, 2, p.E_PAD], f8, isOutput=False)
    d45 = nc.declare_dram_parameter("d45", [45, p.E_PAD], f8, isOutput=False)
    fidx = nc.declare_dram_parameter("fidx", [128, p.T * 8], dt.int16, isOutput=False)
    cid = nc.declare_dram_parameter("cid", [128, p.T], dt.bfloat16, isOutput=False)
    if DIJK_FP8:
        w1a = nc.declare_dram_parameter("w1a", [128, 2, 128], dt.float8e4, isOutput=False)
        w1r = nc.declare_dram_parameter("w1r", [128, 2, 128], dt.float8e5, isOutput=False)
    else:
        w1a = nc.declare_dram_parameter("w1a", [128, 128], dt.bfloat16, isOutput=False)
        w1r = nc.declare_dram_parameter("w1r", [128, 128], dt.bfloat16, isOutput=False)
    w145 = nc.declare_dram_parameter("w145", [45, 128], dt.bfloat16, isOutput=False)
    w2b = nc.declare_dram_parameter("w2b", [128, 128], dt.bfloat16, isOutput=False)
    winb = nc.declare_dram_parameter("winb", [128, 128], dt.bfloat16, isOutput=False)
    woutb = nc.declare_dram_parameter("woutb", [128, 128], dt.bfloat16, isOutput=False)
    wdb = nc.declare_dram_parameter("wdb", [128, 128], dt.bfloat16, isOutput=False)
    iota = nc.declare_dram_parameter("iota", [128, 4, 128], dt.bfloat16, isOutput=False)

    y_out = nc.declare_dram_parameter("y_out", [p.NA_PAD, 128], dt.float32, isOutput=True)
    v_out = nc.declare_dram_parameter("v_out", [p.NA_PAD, 128], dt.float32, isOutput=True)

    # ---- internal dram: feature table (row 0 zero, rows 1.. = xf) ----
    xf_dram = nc.dram_tensor("xf_dram", [p.NXF, 128], dt.bfloat16)

    with tile.TileContext(nc) as tc:
        with (
            tc.tile_pool(name="const", bufs=1) as constp,
            tc.tile_pool(name="xtp", bufs=2) as xtp,
            tc.tile_pool(name="xfp", bufs=2) as xfp,
            tc.tile_pool(name="dld", bufs=2) as dld,
            tc.tile_pool(name="fbp", bufs=3) as fbp,
            tc.tile_pool(name="idxp", bufs=6) as idxp,
            tc.tile_pool(name="eb", bufs=3) as eb,
            tc.tile_pool(name="sgp", bufs=3) as sgp,
            tc.tile_pool(name="tailp", bufs=2) as tailp,
            tc.tile_pool(name="psum", bufs=2, space="PSUM") as psum,
        ):
            # ---- constants ----
            if DIJK_FP8:
                w1a_sb = constp.tile([128, 2, 128], dt.float8e4)
                nc.sync.dma_start(out=w1a_sb[:], in_=w1a[:, :, :])
                w1r_sb = constp.tile([128, 2, 128], dt.float8e5)
                nc.sync.dma_start(out=w1r_sb[:], in_=w1r[:, :, :])
            else:
                w1a_sb = constp.tile([128, 128], dt.bfloat16)
                nc.sync.dma_start(out=w1a_sb[:], in_=w1a[:, :])
                w1r_sb = constp.tile([128, 128], dt.bfloat16)
                nc.sync.dma_start(out=w1r_sb[:], in_=w1r[:, :])
            w145_sb = constp.tile([45, 128], dt.bfloat16)
            nc.sync.dma_start(out=w145_sb[:], in_=w145[:, :])
            w2sb = constp.tile([128, 128], dt.bfloat16)
            nc.sync.dma_start(out=w2sb[:], in_=w2b[:, :])
            winsb = constp.tile([128, 128], dt.bfloat16)
            nc.sync.dma_start(out=winsb[:], in_=winb[:, :])
            woutsb = constp.tile([128, 128], dt.bfloat16)
            nc.sync.dma_start(out=woutsb[:], in_=woutb[:, :])
            wdsb = constp.tile([128, 128], dt.bfloat16)
            nc.sync.dma_start(out=wdsb[:], in_=wdb[:, :])
            iota4_sb = constp.tile([128, 4, 128], dt.bfloat16)
            nc.sync.dma_start(out=iota4_sb[:], in_=iota[:, :, :])
            cid_sb = constp.tile([128, p.T], dt.bfloat16)
            nc.sync.dma_start(out=cid_sb[:], in_=cid[:, :])
            half_c = constp.tile([128, 1], dt.float32)
            nc.gpsimd.memset(half_c[:], 0.5)
            zrow = constp.tile([1, 128], dt.bfloat16)
            nc.gpsimd.memset(zrow[:], 0.0)
            nc.sync.dma_start(out=xf_dram[0:1, :], in_=zrow[:])

            # ---- phase 0: xf = x @ Win -> xf_dram rows 1.. ----
            xts = None
            for sg in range(p.NSG):
                if sg % 4 == 0:
                    wdt = min(2048, p.NXT - sg * 512)
                    xts = xtp.tile([128, 2048], dt.bfloat16, tag="xts")
                    nc.sync.dma_start(
                        out=xts[:, :wdt],
                        in_=xT[:, sg * 512: sg * 512 + wdt],
                    )
                xf_ps = psum.tile([128, 4, 128], dt.float32, tag="z2")
                o = (sg % 4) * 512
                for jj in range(4):
                    nc.tensor.matmul(
                        xf_ps[:, jj, :],
                        xts[:, o + jj * 128: o + (jj + 1) * 128],
                        winsb[:],
                        start=True,
                        stop=True,
                    )
                xf_sb = xfp.tile([128, 4, 128], dt.bfloat16, tag="xfsb")
                nc.vector.tensor_copy(xf_sb[:], xf_ps[:])
                nc.sync.dma_start(
                    out=xf_dram[1 + sg * 512: 1 + sg * 512 + 512, :].rearrange(
                        "(j pp) f -> pp j f", pp=128
                    ),
                    in_=xf_sb[:],
                )

            # ---- tail emitter (per 128-atom chunk) ----
            def emit_tail(c, cps):
                cT = tailp.tile([128, 128], dt.bfloat16, tag="cT")
                nc.vector.tensor_copy(cT[:], cps[:])
                z3_ps = psum.tile([128, 128], dt.float32, tag="tail", bufs=1)
                nc.tensor.matmul(z3_ps[:], woutsb[:], cT[:], start=True, stop=True)
                e3 = tailp.tile([128, 128], dt.float32, tag="e3")
                nc.scalar.activation(e3[:], z3_ps[:], AF.Exp)
                hT = tailp.tile([128, 128], dt.bfloat16, tag="hT")
                nc.scalar.activation(
                    hT[:], e3[:], AF.Ln, bias=half_c[:], scale=half_c[:]
                )
                v_ps = psum.tile([128, 128], dt.float32, tag="tail", bufs=1)
                nc.tensor.matmul(v_ps[:], hT[:], wdsb[:], start=True, stop=True)
                v_sb = tailp.tile([128, 128], dt.float32, tag="v")
                nc.vector.tensor_copy(v_sb[:], v_ps[:])
                nc.sync.dma_start(
                    out=v_out[c * 128:(c + 1) * 128, :], in_=v_sb[:]
                )
                xs = tailp.tile([128, 128], dt.float32, tag="xs")
                nc.sync.dma_start(
                    out=xs[:], in_=xslice[c * 128:(c + 1) * 128, :]
                )
                y_sb = tailp.tile([128, 128], dt.float32, tag="y")
                nc.vector.tensor_tensor(y_sb[:], v_sb[:], xs[:], ALU.add)
                nc.sync.dma_start(
                    out=y_out[c * 128:(c + 1) * 128, :], in_=y_sb[:]
                )

            # ---- edge phase, per chunk ----
            TPC = p.TPC
            NB = _ceil(TPC, 4)                  # 512-edge blocks per chunk
            NPAIR = _ceil(NB, 2)
            gq = 0
            xf_low = xf_dram[0: HALF + 1, :]
            xf_high = xf_dram[HALF + 1: p.NXF, :]

            def mm1(t1_ps, psl, dq, d45t, b, ne):
                e0 = b * 512
                if DIJK_FP8:
                    nc.tensor.matmul(
                        t1_ps[:, psl], w1a_sb[:], dq[:, :, e0:e0 + ne],
                        start=True, stop=False, perf_mode=DR,
                    )
                else:
                    nc.tensor.matmul(
                        t1_ps[:, psl], w1a_sb[:], dq[:, 0, e0:e0 + ne],
                        start=True, stop=False,
                    )
                    nc.tensor.matmul(
                        t1_ps[:, psl], w1r_sb[:], dq[:, 1, e0:e0 + ne],
                        start=False, stop=False,
                    )
                nc.tensor.matmul(
                    t1_ps[:, psl], w145_sb[:], d45t[:, e0:e0 + ne],
                    start=False, stop=True,
                )

            for ci in range(p.NCH):
                e0c = ci * TPC * 128
                dq = dld.tile([128, 2, TPC * 128], f8, tag="dq")
                nc.sync.dma_start(out=dq[:], in_=ddr[:, :, e0c:e0c + TPC * 128])
                d45t = dld.tile([45, TPC * 128], f8, tag="d45")
                nc.sync.dma_start(out=d45t[:], in_=d45[:, e0c:e0c + TPC * 128])
                fgat = fbp.tile([128, TPC, 128], dt.bfloat16, tag="fgat")
                for (t0, nt, is_high) in p.calls:
                    nidx = nt * 128
                    idxt = idxp.tile([128, GCT * 8], dt.int16, tag="idx")
                    col = (ci * TPC + t0) * 8
                    nc.sync.dma_start(
                        out=idxt[:, : nt * 8], in_=fidx[:, col: col + nt * 8]
                    )
                    nc.gpsimd.dma_gather(
                        fgat[:, t0:t0 + nt, :],
                        xf_high if is_high else xf_low,
                        idxt[:, : nt * 8], nidx, nidx, 128,
                        single_packet=True, queue_num=gq % 4,
                    )
                    gq += 1

                conv_ps = None
                for pr in range(NPAIR):
                    b0 = pr * 2
                    nblk = min(2, NB - b0)
                    ntl = [min(4, TPC - 4 * (b0 + i)) for i in range(nblk)]
                    ntt = sum(ntl)
                    # mm1 for the pair
                    t1_ps = psum.tile([128, 1024], dt.float32, tag="t1")
                    for i in range(nblk):
                        mm1(t1_ps, slice(i * 512, i * 512 + ntl[i] * 128),
                            dq, d45t, b0 + i, ntl[i] * 128)
                    # Exp over the pair (from PSUM), Ln over the pair
                    e1q = eb.tile([128, 8, 128], dt.float32, tag="e1")
                    t1sq = eb.tile([128, 8, 128], dt.bfloat16, tag="t1s")
                    lnsl = (
                        slice(0, 4 + ntl[1]) if nblk == 2 and ntl[0] == 4
                        else slice(0, ntl[0])
                    )
                    nc.scalar.activation(
                        e1q[:, lnsl, :],
                        t1_ps[:, : ntt * 128].rearrange(
                            "pp (i f) -> pp i f", f=128
                        ),
                        AF.Exp,
                    )
                    nc.scalar.activation(
                        t1sq[:, lnsl, :], e1q[:, lnsl, :], AF.Ln,
                        bias=half_c[:], scale=half_c[:],
                    )
                    # mm2 + Exp per block, Ln per pair
                    e2q = eb.tile([128, 8, 128], dt.float32, tag="e2")
                    wq = eb.tile([128, 8, 128], dt.bfloat16, tag="w")
                    for i in range(nblk):
                        z2_ps = psum.tile([128, 4, 128], dt.float32, tag="z2")
                        for t in range(ntl[i]):
                            nc.tensor.matmul(
                                z2_ps[:, t, :],
                                t1sq[:, i * 4 + t, :],
                                w2sb[:],
                                start=True, stop=True,
                            )
                        nc.scalar.activation(
                            e2q[:, i * 4: i * 4 + ntl[i], :],
                            z2_ps[:, : ntl[i], :],
                            AF.Exp,
                        )
                    nc.scalar.activation(
                        wq[:, lnsl, :], e2q[:, lnsl, :], AF.Ln,
                        bias=half_c[:], scale=half_c[:],
                    )
                    # wf, S, conv accumulation
                    for i in range(nblk):
                        b = b0 + i
                        nt = ntl[i]
                        t0 = b * 4
                        wf = sgp.tile([128, 4, 128], dt.bfloat16, tag="wf")
                        nc.vector.tensor_tensor(
                            wf[:, :nt, :],
                            wq[:, i * 4: i * 4 + nt, :],
                            fgat[:, t0:t0 + nt, :],
                            ALU.mult,
                        )
                        S_blk = sgp.tile([128, 4, 128], dt.bfloat16, tag="S")
                        nc.vector.tensor_tensor(
                            S_blk[:, :nt, :],
                            iota4_sb[:, :nt, :],
                            cid_sb[:, ci * TPC + t0: ci * TPC + t0 + nt]
                            .to_broadcast([128, nt, 128]),
                            ALU.is_equal,
                        )
                        for t in range(nt):
                            k = t0 + t
                            if k == 0:
                                conv_ps = psum.tile(
                                    [128, 128], dt.float32, tag="conv", bufs=1
                                )
                            nc.tensor.matmul(
                                conv_ps[:], wf[:, t, :], S_blk[:, t, :],
                                start=(k == 0), stop=(k == TPC - 1),
                            )
                emit_tail(ci, conv_ps)

    nc.finalize()
    return nc


_PROG_CACHE = {}


def kernel(x, dijk, W1, b1, W2, b2, Win, Wout, bout, Wd, bd, idx_j, seg_i, seg_j):
    x = np.ascontiguousarray(np.asarray(x, dtype=np.float32))
    for b in (b1, b2, bout, bd):
        assert np.abs(np.asarray(b)).max() == 0.0, "nonzero biases unsupported"

    n_atoms, n_basis = x.shape
    n_edges, n_in = np.asarray(dijk).shape
    assert n_basis == 128 and np.asarray(W2).shape == (128, 128)

    p, bounds = _plan_from_data(n_atoms, n_edges, n_in, idx_j, seg_i)
    per_core = shard_inputs(p, dijk, idx_j, seg_i, x, bounds)

    key = (n_atoms, n_edges, n_in, p.Lmax, p.Hmax, DIJK_FP8)
    if key not in _PROG_CACHE:
        _PROG_CACHE[key] = build_program(p)
    nc = _PROG_CACHE[key]

    W1f = np.asarray(W1, dtype=np.float32)
    if DIJK_FP8:
        w1hi = W1f[:256].astype(E4M3)
        w1res = (W1f[:256] - w1hi.astype(np.float32)).astype(E5M2)
        w1a_h = np.ascontiguousarray(w1hi.reshape(128, 2, 128))
        w1r_h = np.ascontiguousarray(w1res.reshape(128, 2, 128))
    else:
        w1a_h = np.ascontiguousarray(W1f[0:128].astype(BF16))
        w1r_h = np.ascontiguousarray(W1f[128:256].astype(BF16))
    w145_h = np.zeros((45, 128), dtype=BF16)
    w145_h[:44] = W1f[256:300].astype(BF16)
    w145_h[44] = (0.5 * W1f.sum(axis=0)).astype(BF16)

    xTh = np.zeros((128, p.NXT), dtype=BF16)
    xTh[:, :n_atoms] = x.T
    common = dict(
        xT=xTh,
        w1a=w1a_h,
        w1r=w1r_h,
        w145=w145_h,
        w2b=np.asarray(W2, dtype=np.float32).astype(BF16),
        winb=np.asarray(Win, dtype=np.float32).astype(BF16),
        woutb=np.asarray(Wout, dtype=np.float32).astype(BF16),
        wdb=np.asarray(Wd, dtype=np.float32).astype(BF16),
        iota=np.tile(np.arange(128, dtype=np.float32).astype(BF16), (128, 4, 1)),
    )
    in_maps = [{**common, **pc} for pc in per_core]
    res = run_bass_kernel_spmd(nc, in_maps, list(range(N_CORES)))
    global LAST_RESULTS
    LAST_RESULTS = res

    y = np.empty((n_atoms, 128), dtype=np.float32)
    v = np.empty((n_atoms, 128), dtype=np.float32)
    for c in range(N_CORES):
        y[c * p.NA:(c + 1) * p.NA] = res.results[c]["y_out"][: p.NA]
        v[c * p.NA:(c + 1) * p.NA] = res.results[c]["v_out"][: p.NA]
    return (y, v)


# revision 10
# speedup vs baseline: 1.2567x; 1.0180x over previous
"""CFNet interaction block on 8 trn2 NeuronCores — v3 SPMD bass/tile kernel.

Strategy (per core c of 8, SPMD — one program, per-core input data):
  - Edges sharded by ATOM ranges: core c owns atoms [c*NA, (c+1)*NA) and all
    edges whose (sorted) seg_i falls there.  Outputs are disjoint; the host
    concatenates (no device collective).
  - Edges grouped by 128-atom chunk of seg_i.  Within a chunk, edges are
    reordered [idx_j < 25000 ..., idx_j >= 25000 ...], each run padded to a
    UNIFORM tile count (Lmax / Hmax tiles of 128 edges, data-derived max over
    all cores+chunks).  Padding slots are dump edges: dijk-cols = 0 ->
    w = ssp(ssp(0)@W2) = 0 exactly, and their gather idx points at a zeroed
    row of the feature table, so they contribute nothing.
  - The low/high split keeps SWDGE gather indices within int16: the feature
    table xf_dram [50177, 128] bf16 holds row 0 = zeros, rows 1..50000 =
    xf = x @ Win (computed on device), rows 50001.. = zeros.  Low calls
    gather rows [0..25000] (idx = j+1), high calls gather from the slice
    starting at row 25001 (idx = j-25000; zero pad idx = 25000).  256B/row
    single-packet gathers - half the HBM traffic of the old 512B pair
    gathers, and no DVE parity-select is needed.
  - dijk is uploaded centered and fp8: ddr [128, 2, E_PAD] float8_e4m3 holds
    (dijk - 0.5) for k = 0..255 in DoubleRow operand layout (k = 2p+r), and
    d45 [45, E_PAD] holds k = 256..299 plus a constant-ones row.  mm1 =
    2 DoubleRow matmuls (fp8 hi + e5m2 residual of W1) + one 45-row bf16
    matmul whose last row adds 0.5*colsum(W1), undoing the centering
    exactly.  Centering halves the fp8 quantization error of dijk.
  - Per pair of 512-edge blocks: mm1 -> Exp (per 1024-edge PSUM pair-tile),
    Ln(0.5 + 0.5 e) batched per 2048-edge quad; mm2 per 128-edge tile;
    Exp per pair, Ln per quad -> w; wf = w * f (DVE, bf16); one-hot S via
    iota == cid (DVE); conv^T[f,a] += wf_t^T @ S_t accumulated in PSUM over
    the chunk's TPC tiles.
  - Tail per chunk (pipelined): z3^T = Wout^T conv, ssp, v = h @ Wd,
    y = x + v.
"""

import math
import sys

import numpy as np
import ml_dtypes

sys.path.insert(0, "/opt/trn_rl_repo")

import concourse.bacc as bacc
import concourse.bass as bass
import concourse.mybir as mybir
from concourse import tile
from concourse.bass_utils import run_bass_kernel_spmd

dt = mybir.dt
AF = mybir.ActivationFunctionType
ALU = mybir.AluOpType
BF16 = ml_dtypes.bfloat16
E4M3 = mybir.dt.np(dt.float8e4)
E5M2 = mybir.dt.np(dt.float8e5)

N_CORES = 8
ACH = 128            # atoms per conv chunk
HALF = 25000         # atom-id split for int16 gather range
GCT = 8              # max tiles (x128 idx) per dma_gather call (ring = 1024)
DIJK_FP8 = True


def _ceil(a, b):
    return -(-a // b)


class Plan:
    def __init__(self, n_atoms, n_edges, n_in, lmax, hmax):
        assert n_atoms % N_CORES == 0 and n_in == 300
        self.n_atoms, self.n_edges, self.n_in = n_atoms, n_edges, n_in
        self.NA = n_atoms // N_CORES
        self.NCH = _ceil(self.NA, ACH)
        self.NA_PAD = self.NCH * ACH
        self.Lmax, self.Hmax = lmax, hmax
        self.TPC = lmax + hmax
        self.T = self.NCH * self.TPC
        self.E_PAD = self.T * 128
        # xf table: row 0 zero, rows 1..n_atoms = atoms, padding zero rows
        self.NSG = _ceil(n_atoms, 512)            # write groups of 512 rows
        self.NXF = 1 + self.NSG * 512             # xf_dram rows
        self.NXT = self.NSG * 512                 # xT columns
        # gather call layout within a chunk: (row_off, ntiles, is_high)
        calls = []
        for t0 in range(0, lmax, GCT):
            calls.append((t0, min(GCT, lmax - t0), 0))
        for t0 in range(0, hmax, GCT):
            calls.append((lmax + t0, min(GCT, hmax - t0), 1))
        self.calls = calls


def _plan_from_data(n_atoms, n_edges, n_in, idx_j, seg_i):
    na = n_atoms // N_CORES
    nch = _ceil(na, ACH)
    seg = np.asarray(seg_i).astype(np.int64)
    idxj = np.asarray(idx_j).astype(np.int64)
    bounds = np.searchsorted(seg, np.arange(N_CORES + 1) * na)
    lmax = hmax = 1
    for c in range(N_CORES):
        lo, hi = int(bounds[c]), int(bounds[c + 1])
        if lo == hi:
            continue
        ch = (seg[lo:hi] - c * na) // ACH
        high = (idxj[lo:hi] >= HALF).astype(np.int64)
        cnt = np.bincount(ch * 2 + high, minlength=nch * 2)
        lmax = max(lmax, _ceil(int(cnt[0::2].max()), 128))
        hmax = max(hmax, _ceil(int(cnt[1::2].max()), 128))
    return Plan(n_atoms, n_edges, n_in, lmax, hmax), bounds


def _wrap_idx(idx1d):
    """[n] int -> [128, n//16] int16 SWDGE layout (16-wrapped, tiled x8)."""
    n = idx1d.shape[0]
    w = idx1d.astype(np.int16).reshape(n // 16, 16).T
    return np.tile(w, (8, 1))


def shard_inputs(p, dijk, idx_j, seg_i, x, bounds):
    seg = np.asarray(seg_i).astype(np.int64)
    idxj = np.asarray(idx_j).astype(np.int64)
    n_in = p.n_in
    per_core = []
    for c in range(N_CORES):
        lo, hi = int(bounds[c]), int(bounds[c + 1])
        es = seg[lo:hi] - c * p.NA
        ej = idxj[lo:hi]
        n = hi - lo
        ch = es // ACH
        high = (ej >= HALF).astype(np.int64)
        gid = ch * 2 + high
        perm = np.argsort(gid, kind="stable")
        gid_s = gid[perm]
        sizes = np.bincount(gid, minlength=p.NCH * 2)
        first = np.zeros(p.NCH * 2, dtype=np.int64)
        first[1:] = np.cumsum(sizes)[:-1]
        base = np.empty(p.NCH * 2, dtype=np.int64)
        base[0::2] = np.arange(p.NCH) * p.TPC * 128
        base[1::2] = np.arange(p.NCH) * p.TPC * 128 + p.Lmax * 128
        dst = base[gid_s] + (np.arange(n) - first[gid_s])

        es_s, ej_s, high_s = es[perm], ej[perm], high[perm]

        # dijk stream, centered, with const row
        dsrc = np.asarray(dijk[lo:hi], dtype=np.float32)[perm] - 0.5
        if DIJK_FP8:
            dstream = np.zeros((p.E_PAD, 256), dtype=E4M3)
            dstream[dst] = dsrc[:, :256].astype(E4M3)
            ddr = np.ascontiguousarray(
                dstream.reshape(p.E_PAD, 128, 2).transpose(1, 2, 0)
            )  # [128, 2, E_PAD]: (p, r) -> k = 2p + r
            d45 = np.zeros((45, p.E_PAD), dtype=E4M3)
            d45[:44, dst] = dsrc[:, 256:300].astype(E4M3).T
            d45[44, dst] = np.ones(n, dtype=E4M3)
        else:
            dstream = np.zeros((p.E_PAD, 256), dtype=BF16)
            dstream[dst] = dsrc[:, :256].astype(BF16)
            ddr = np.ascontiguousarray(
                dstream.reshape(p.E_PAD, 2, 128).transpose(2, 1, 0)
            )  # [128, 2, E_PAD]: (p, s) -> k = s*128 + p
            d45 = np.zeros((45, p.E_PAD), dtype=BF16)
            d45[:44, dst] = dsrc[:, 256:300].astype(BF16).T
            d45[44, dst] = np.ones(n, dtype=BF16)
        del dstream, dsrc

        cidv = np.zeros(p.E_PAD, dtype=np.float32)
        cidv[dst] = (es_s % ACH).astype(np.float32)
        cid_tbl = np.ascontiguousarray(
            cidv.reshape(p.T, 128).T.astype(BF16)
        )  # [128, T]

        gv = np.zeros((p.NCH, p.TPC, 128), dtype=np.int64)
        gv[:, p.Lmax:, :] = HALF            # high-region padding -> zero row
        gv = gv.reshape(-1)
        gv[dst[high_s == 0]] = ej_s[high_s == 0] + 1
        gv[dst[high_s == 1]] = ej_s[high_s == 1] - HALF
        fidx = np.zeros((128, p.T * 8), dtype=np.int16)
        for ci in range(p.NCH):
            for (t0, nt, _hi) in p.calls:
                s0 = (ci * p.TPC + t0) * 128
                col = (ci * p.TPC + t0) * 8
                fidx[:, col: col + nt * 8] = _wrap_idx(gv[s0: s0 + nt * 128])

        per_core.append(
            dict(
                ddr=ddr,
                d45=np.ascontiguousarray(d45),
                cid=cid_tbl,
                fidx=fidx,
                xslice=_pad_rows(x[c * p.NA:(c + 1) * p.NA], p.NA_PAD),
            )
        )
    return per_core


def _pad_rows(a, n):
    out = np.zeros((n,) + a.shape[1:], dtype=np.asarray(a).dtype)
    out[: a.shape[0]] = np.asarray(a)
    return out


def build_program(p):
    # Force one activation table (Exp+Ln coexist in natural_log_exp_and_others)
    import concourse.bacc as _bacc_mod
    _orig_gat = _bacc_mod.get_activation_tables

    def _one_table(arch):
        t = _orig_gat(arch)
        keep = "natural_log_exp_and_others"
        assert keep in t
        return {k: (v if k == keep else set()) for k, v in t.items()}

    _bacc_mod.get_activation_tables = _one_table
    try:
        return _build_program_inner(p)
    finally:
        _bacc_mod.get_activation_tables = _orig_gat


def _build_program_inner(p):
    nc = bacc.Bacc(None, target_bir_lowering=False, num_swdge_queues=4)
    DR = mybir.MatmulPerfMode.DoubleRow
    f8 = dt.float8e4 if DIJK_FP8 else dt.bfloat16

    # ---- dram parameters ----
    xT = nc.declare_dram_parameter("xT", [128, p.NXT], dt.bfloat16, isOutput=False)
    xslice = nc.declare_dram_parameter("xslice", [p.NA_PAD, 128], dt.float32, isOutput=False)
    ddr = nc.declare_dram_parameter("ddr", [128, 2, p.E_PAD], f8, isOutput=False)
    d45 = nc.declare_dram_parameter("d45", [45, p.E_PAD], f8, isOutput=False)
    fidx = nc.declare_dram_parameter("fidx", [128, p.T * 8], dt.int16, isOutput=False)
    cid = nc.declare_dram_parameter("cid", [128, p.T], dt.bfloat16, isOutput=False)
    if DIJK_FP8:
        w1a = nc.declare_dram_parameter("w1a", [128, 2, 128], dt.float8e4, isOutput=False)
        w1r = nc.declare_dram_parameter("w1r", [128, 2, 128], dt.float8e5, isOutput=False)
    else:
        w1a = nc.declare_dram_parameter("w1a", [128, 128], dt.bfloat16, isOutput=False)
        w1r = nc.declare_dram_parameter("w1r", [128, 128], dt.bfloat16, isOutput=False)
    w145 = nc.declare_dram_parameter("w145", [45, 128], dt.bfloat16, isOutput=False)
    w2b = nc.declare_dram_parameter("w2b", [128, 128], dt.bfloat16, isOutput=False)
    winb = nc.declare_dram_parameter("winb", [128, 128], dt.bfloat16, isOutput=False)
    woutb = nc.declare_dram_parameter("woutb", [128, 128], dt.bfloat16, isOutput=False)
    wdb = nc.declare_dram_parameter("wdb", [128, 128], dt.bfloat16, isOutput=False)
    iota = nc.declare_dram_parameter("iota", [128, 4, 128], dt.bfloat16, isOutput=False)

    y_out = nc.declare_dram_parameter("y_out", [p.NA_PAD, 128], dt.float32, isOutput=True)
    v_out = nc.declare_dram_parameter("v_out", [p.NA_PAD, 128], dt.float32, isOutput=True)

    # ---- internal dram: feature table (row 0 zero, rows 1.. = xf) ----
    xf_dram = nc.dram_tensor("xf_dram", [p.NXF, 128], dt.bfloat16)

    with tile.TileContext(nc) as tc:
        with (
            tc.tile_pool(name="const", bufs=1) as constp,
            tc.tile_pool(name="xtp", bufs=2) as xtp,
            tc.tile_pool(name="xfp", bufs=2) as xfp,
            tc.tile_pool(name="dld", bufs=3) as dld,
            tc.tile_pool(name="fbp", bufs=3) as fbp,
            tc.tile_pool(name="idxp", bufs=12) as idxp,
            tc.tile_pool(name="eb", bufs=3) as eb,
            tc.tile_pool(name="sgp", bufs=3) as sgp,
            tc.tile_pool(name="tailp", bufs=2) as tailp,
            tc.tile_pool(name="psum", bufs=2, space="PSUM") as psum,
        ):
            # ---- constants ----
            if DIJK_FP8:
                w1a_sb = constp.tile([128, 2, 128], dt.float8e4)
                nc.sync.dma_start(out=w1a_sb[:], in_=w1a[:, :, :])
                w1r_sb = constp.tile([128, 2, 128], dt.float8e5)
                nc.sync.dma_start(out=w1r_sb[:], in_=w1r[:, :, :])
            else:
                w1a_sb = constp.tile([128, 128], dt.bfloat16)
                nc.sync.dma_start(out=w1a_sb[:], in_=w1a[:, :])
                w1r_sb = constp.tile([128, 128], dt.bfloat16)
                nc.sync.dma_start(out=w1r_sb[:], in_=w1r[:, :])
            w145_sb = constp.tile([45, 128], dt.bfloat16)
            nc.sync.dma_start(out=w145_sb[:], in_=w145[:, :])
            w2sb = constp.tile([128, 128], dt.bfloat16)
            nc.sync.dma_start(out=w2sb[:], in_=w2b[:, :])
            winsb = constp.tile([128, 128], dt.bfloat16)
            nc.sync.dma_start(out=winsb[:], in_=winb[:, :])
            woutsb = constp.tile([128, 128], dt.bfloat16)
            nc.sync.dma_start(out=woutsb[:], in_=woutb[:, :])
            wdsb = constp.tile([128, 128], dt.bfloat16)
            nc.sync.dma_start(out=wdsb[:], in_=wdb[:, :])
            iota4_sb = constp.tile([128, 4, 128], dt.bfloat16)
            nc.sync.dma_start(out=iota4_sb[:], in_=iota[:, :, :])
            cid_sb = constp.tile([128, p.T], dt.bfloat16)
            nc.sync.dma_start(out=cid_sb[:], in_=cid[:, :])
            half_c = constp.tile([128, 1], dt.float32)
            nc.gpsimd.memset(half_c[:], 0.5)
            zrow = constp.tile([1, 128], dt.bfloat16)
            nc.gpsimd.memset(zrow[:], 0.0)
            nc.sync.dma_start(out=xf_dram[0:1, :], in_=zrow[:])

            # ---- edge phase, per chunk ----
            TPC = p.TPC
            NB = _ceil(TPC, 4)                  # 512-edge blocks per chunk
            NPAIR = _ceil(NB, 2)
            gq = 0
            xf_low = xf_dram[0: HALF + 1, :]
            xf_high = xf_dram[HALF + 1: p.NXF, :]

            def mm1(t1_ps, psl, dq, d45t, b, ne):
                e0 = b * 512
                if DIJK_FP8:
                    nc.tensor.matmul(
                        t1_ps[:, psl], w1a_sb[:], dq[:, :, e0:e0 + ne],
                        start=True, stop=False, perf_mode=DR,
                    )
                else:
                    nc.tensor.matmul(
                        t1_ps[:, psl], w1a_sb[:], dq[:, 0, e0:e0 + ne],
                        start=True, stop=False,
                    )
                    nc.tensor.matmul(
                        t1_ps[:, psl], w1r_sb[:], dq[:, 1, e0:e0 + ne],
                        start=False, stop=False,
                    )
                nc.tensor.matmul(
                    t1_ps[:, psl], w145_sb[:], d45t[:, e0:e0 + ne],
                    start=False, stop=True,
                )

            def chunk_loads(ci):
                e0c = ci * TPC * 128
                dq = dld.tile([128, 2, TPC * 128], f8, tag="dq", name=f"dq{ci}")
                nc.sync.dma_start(out=dq[:], in_=ddr[:, :, e0c:e0c + TPC * 128])
                d45t = dld.tile([45, TPC * 128], f8, tag="d45", name=f"d45t{ci}")
                nc.sync.dma_start(out=d45t[:], in_=d45[:, e0c:e0c + TPC * 128])
                idxts = []
                for (t0, nt, is_high) in p.calls:
                    idxt = idxp.tile([128, GCT * 8], dt.int16, tag="idx",
                                     name=f"idxt{ci}_{t0}")
                    col = (ci * TPC + t0) * 8
                    nc.sync.dma_start(
                        out=idxt[:, : nt * 8], in_=fidx[:, col: col + nt * 8]
                    )
                    idxts.append(idxt)
                return dq, d45t, idxts

            def chunk_gathers(ci, idxts):
                nonlocal gq
                fgat = fbp.tile([128, TPC, 128], dt.bfloat16, tag="fgat",
                                name=f"fgat{ci}")
                for (t0, nt, is_high), idxt in zip(p.calls, idxts):
                    nidx = nt * 128
                    nc.gpsimd.dma_gather(
                        fgat[:, t0:t0 + nt, :],
                        xf_high if is_high else xf_low,
                        idxt[:, : nt * 8], nidx, nidx, 128,
                        single_packet=True, queue_num=gq % 4,
                    )
                    gq += 1
                return fgat

            gt = {}

            PF = 2
            ld = {}
            for _ci in range(min(PF + 1, p.NCH)):
                ld[_ci] = chunk_loads(_ci)

            # ---- phase 0: xf = x @ Win -> xf_dram rows 1.. ----
            xts = None
            for sg in range(p.NSG):
                if sg % 4 == 0:
                    wdt = min(2048, p.NXT - sg * 512)
                    xts = xtp.tile([128, 2048], dt.bfloat16, tag="xts")
                    nc.sync.dma_start(
                        out=xts[:, :wdt],
                        in_=xT[:, sg * 512: sg * 512 + wdt],
                    )
                xf_ps = psum.tile([128, 4, 128], dt.float32, tag="t1")
                o = (sg % 4) * 512
                for jj in range(4):
                    nc.tensor.matmul(
                        xf_ps[:, jj, :],
                        xts[:, o + jj * 128: o + (jj + 1) * 128],
                        winsb[:],
                        start=True,
                        stop=True,
                    )
                xf_sb = xfp.tile([128, 4, 128], dt.bfloat16, tag="xfsb")
                nc.vector.tensor_copy(xf_sb[:], xf_ps[:])
                nc.sync.dma_start(
                    out=xf_dram[1 + sg * 512: 1 + sg * 512 + 512, :].rearrange(
                        "(j pp) f -> pp j f", pp=128
                    ),
                    in_=xf_sb[:],
                )

            # ---- tail emitter (per 128-atom chunk) ----
            def emit_tail(c, cps):
                cT = tailp.tile([128, 128], dt.bfloat16, tag="cT")
                nc.vector.tensor_copy(cT[:], cps[:])
                z3_ps = psum.tile([128, 128], dt.float32, tag="tail", bufs=1)
                nc.tensor.matmul(z3_ps[:], woutsb[:], cT[:], start=True, stop=True)
                e3 = tailp.tile([128, 128], dt.float32, tag="e3")
                nc.scalar.activation(e3[:], z3_ps[:], AF.Exp)
                hT = tailp.tile([128, 128], dt.bfloat16, tag="hT")
                nc.scalar.activation(
                    hT[:], e3[:], AF.Ln, bias=half_c[:], scale=half_c[:]
                )
                v_ps = psum.tile([128, 128], dt.float32, tag="tail", bufs=1)
                nc.tensor.matmul(v_ps[:], hT[:], wdsb[:], start=True, stop=True)
                v_sb = tailp.tile([128, 128], dt.float32, tag="v")
                nc.vector.tensor_copy(v_sb[:], v_ps[:])
                nc.sync.dma_start(
                    out=v_out[c * 128:(c + 1) * 128, :], in_=v_sb[:]
                )
                xs = tailp.tile([128, 128], dt.float32, tag="xs")
                nc.sync.dma_start(
                    out=xs[:], in_=xslice[c * 128:(c + 1) * 128, :]
                )
                y_sb = tailp.tile([128, 128], dt.float32, tag="y")
                nc.vector.tensor_tensor(y_sb[:], v_sb[:], xs[:], ALU.add)
                nc.sync.dma_start(
                    out=y_out[c * 128:(c + 1) * 128, :], in_=y_sb[:]
                )

            # ---- edge phase main loop ----
            for ci in range(p.NCH):
                dq, d45t, idxts = ld.pop(ci)
                if ci not in gt:
                    gt[ci] = chunk_gathers(ci, idxts)
                fgat = gt.pop(ci)

                conv_ps = None
                for pr in range(NPAIR):
                    b0 = pr * 2
                    nblk = min(2, NB - b0)
                    ntl = [min(4, TPC - 4 * (b0 + i)) for i in range(nblk)]
                    ntt = sum(ntl)
                    # mm1 for the pair
                    t1_ps = psum.tile([128, 1024], dt.float32, tag="t1")
                    for i in range(nblk):
                        mm1(t1_ps, slice(i * 512, i * 512 + ntl[i] * 128),
                            dq, d45t, b0 + i, ntl[i] * 128)
                    # Exp over the pair (from PSUM), Ln over the pair
                    e1q = eb.tile([128, 8, 128], dt.float32, tag="e1")
                    t1sq = eb.tile([128, 8, 128], dt.bfloat16, tag="t1s")
                    lnsl = (
                        slice(0, 4 + ntl[1]) if nblk == 2 and ntl[0] == 4
                        else slice(0, ntl[0])
                    )
                    nc.scalar.activation(
                        e1q[:, lnsl, :],
                        t1_ps[:, : ntt * 128].rearrange(
                            "pp (i f) -> pp i f", f=128
                        ),
                        AF.Exp,
                    )
                    nc.scalar.activation(
                        t1sq[:, lnsl, :], e1q[:, lnsl, :], AF.Ln,
                        bias=half_c[:], scale=half_c[:],
                    )
                    # mm2 + Exp per block, Ln per pair
                    e2q = eb.tile([128, 8, 128], dt.float32, tag="e2")
                    wq = eb.tile([128, 8, 128], dt.bfloat16, tag="w")
                    z2_ps = psum.tile([128, 8, 128], dt.float32, tag="z2", bufs=1)
                    for i in range(nblk):
                        for t in range(ntl[i]):
                            nc.tensor.matmul(
                                z2_ps[:, i * 4 + t, :],
                                t1sq[:, i * 4 + t, :],
                                w2sb[:],
                                start=True, stop=True,
                            )
                    nc.scalar.activation(
                        e2q[:, lnsl, :], z2_ps[:, lnsl, :], AF.Exp,
                    )
                    nc.scalar.activation(
                        wq[:, lnsl, :], e2q[:, lnsl, :], AF.Ln,
                        bias=half_c[:], scale=half_c[:],
                    )
                    # wf, S, conv accumulation
                    for i in range(nblk):
                        b = b0 + i
                        nt = ntl[i]
                        t0 = b * 4
                        wf = sgp.tile([128, 4, 128], dt.bfloat16, tag="wf")
                        nc.vector.tensor_tensor(
                            wf[:, :nt, :],
                            wq[:, i * 4: i * 4 + nt, :],
                            fgat[:, t0:t0 + nt, :],
                            ALU.mult,
                        )
                        S_blk = sgp.tile([128, 4, 128], dt.bfloat16, tag="S")
                        nc.vector.tensor_tensor(
                            S_blk[:, :nt, :],
                            iota4_sb[:, :nt, :],
                            cid_sb[:, ci * TPC + t0: ci * TPC + t0 + nt]
                            .to_broadcast([128, nt, 128]),
                            ALU.is_equal,
                        )
                        for t in range(nt):
                            k = t0 + t
                            if k == 0:
                                conv_ps = psum.tile(
                                    [128, 128], dt.float32, tag="conv", bufs=1
                                )
                            nc.tensor.matmul(
                                conv_ps[:], wf[:, t, :], S_blk[:, t, :],
                                start=(k == 0), stop=(k == TPC - 1),
                            )
                emit_tail(ci, conv_ps)
                if ci + 1 < p.NCH and (ci + 1) not in gt:
                    gt[ci + 1] = chunk_gathers(ci + 1, ld[ci + 1][2])
                if ci + PF + 1 < p.NCH:
                    ld[ci + PF + 1] = chunk_loads(ci + PF + 1)

    nc.finalize()
    return nc


_PROG_CACHE = {}


def kernel(x, dijk, W1, b1, W2, b2, Win, Wout, bout, Wd, bd, idx_j, seg_i, seg_j):
    x = np.ascontiguousarray(np.asarray(x, dtype=np.float32))
    for b in (b1, b2, bout, bd):
        assert np.abs(np.asarray(b)).max() == 0.0, "nonzero biases unsupported"

    n_atoms, n_basis = x.shape
    n_edges, n_in = np.asarray(dijk).shape
    assert n_basis == 128 and np.asarray(W2).shape == (128, 128)

    p, bounds = _plan_from_data(n_atoms, n_edges, n_in, idx_j, seg_i)
    per_core = shard_inputs(p, dijk, idx_j, seg_i, x, bounds)

    key = (n_atoms, n_edges, n_in, p.Lmax, p.Hmax, DIJK_FP8)
    if key not in _PROG_CACHE:
        _PROG_CACHE[key] = build_program(p)
    nc = _PROG_CACHE[key]

    W1f = np.asarray(W1, dtype=np.float32)
    if DIJK_FP8:
        w1hi = W1f[:256].astype(E4M3)
        w1res = (W1f[:256] - w1hi.astype(np.float32)).astype(E5M2)
        w1a_h = np.ascontiguousarray(w1hi.reshape(128, 2, 128))
        w1r_h = np.ascontiguousarray(w1res.reshape(128, 2, 128))
    else:
        w1a_h = np.ascontiguousarray(W1f[0:128].astype(BF16))
        w1r_h = np.ascontiguousarray(W1f[128:256].astype(BF16))
    w145_h = np.zeros((45, 128), dtype=BF16)
    w145_h[:44] = W1f[256:300].astype(BF16)
    w145_h[44] = (0.5 * W1f.sum(axis=0)).astype(BF16)

    xTh = np.zeros((128, p.NXT), dtype=BF16)
    xTh[:, :n_atoms] = x.T
    common = dict(
        xT=xTh,
        w1a=w1a_h,
        w1r=w1r_h,
        w145=w145_h,
        w2b=np.asarray(W2, dtype=np.float32).astype(BF16),
        winb=np.asarray(Win, dtype=np.float32).astype(BF16),
        woutb=np.asarray(Wout, dtype=np.float32).astype(BF16),
        wdb=np.asarray(Wd, dtype=np.float32).astype(BF16),
        iota=np.tile(np.arange(128, dtype=np.float32).astype(BF16), (128, 4, 1)),
    )
    in_maps = [{**common, **pc} for pc in per_core]
    res = run_bass_kernel_spmd(nc, in_maps, list(range(N_CORES)))
    global LAST_RESULTS
    LAST_RESULTS = res

    y = np.empty((n_atoms, 128), dtype=np.float32)
    v = np.empty((n_atoms, 128), dtype=np.float32)
    for c in range(N_CORES):
        y[c * p.NA:(c + 1) * p.NA] = res.results[c]["y_out"][: p.NA]
        v[c * p.NA:(c + 1) * p.NA] = res.results[c]["v_out"][: p.NA]
    return (y, v)
